# revision 1
# baseline (speedup 1.0000x reference)
"""Dual-path multi-head attention on 8 trn2 NeuronCores.

Sharding: core c = (path p=c//4, batch b=c%4). Each core runs the full
pipeline for one path and one batch element: 3 input projections, 16-head
attention (S=1024, dh=64), output projection. No collectives.

Path 2 cross-wiring (q2 from k; k2,v2 from q) is handled purely by host-side
input routing - every core runs the identical SPMD program.

Device layouts (per core, all pre-packed on host for contiguous DMA runs):
  xq/xk/xv : [p, n, s]   = x.T blocked:  x[s, n*128+p]
  wq/wc    : [p, m, n, e'] = W[m*128+e', n*128+p]  (W.T blocked by out-block m)
  wv       : [p, n, e]   = Wv[e, n*128+p]
  Projections compute Q1T/K1T = [e, s] and V1 = [s, e]; scores are computed
  transposed (probs_T[sk, sq]) so softmax needs no transposes. Softmax is
  max-free (scores ~ N(0,1)); the denominator comes from a ones-column
  appended per head slot in v1e (PV yields [dh+1, sq], row 64 = sum).
Emission order software-pipelines Q/K projections of block m+1 between the
two attention heads of block m so their PSUM->SBUF drains hide under PE work.
"""

import numpy as np
import ml_dtypes

B, S, D, H, DH = 4, 1024, 1024, 16, 64
NB = D // 128  # 8 partition-blocks
HW = 65  # head slot width in v1e (64 data + 1 ones col)

_compiled = None


def _build():
    import concourse.bass as bass
    import concourse.mybir as mybir
    import concourse.tile as tile
    from concourse import bacc

    dt = mybir.dt
    f32, bf16, f32r = dt.float32, dt.bfloat16, dt.float32r

    nc = bacc.Bacc("TRN2", target_bir_lowering=False, debug=False)

    xq_d = nc.dram_tensor("xq", [128, NB, S], bf16, kind="ExternalInput")
    xk_d = nc.dram_tensor("xk", [128, NB, S], bf16, kind="ExternalInput")
    xv_d = nc.dram_tensor("xv", [128, NB, S], bf16, kind="ExternalInput")
    wq_d = nc.dram_tensor("wq", [128, NB, NB, 128], bf16, kind="ExternalInput")
    wk_d = nc.dram_tensor("wk", [128, NB, NB, 128], bf16, kind="ExternalInput")
    wv_d = nc.dram_tensor("wv", [128, NB, D], bf16, kind="ExternalInput")
    wc_d = nc.dram_tensor("wc", [128, NB, NB, 128], bf16, kind="ExternalInput")
    bq_d = nc.dram_tensor("bq", [128, NB], f32, kind="ExternalInput")
    bk_d = nc.dram_tensor("bk", [128, NB], f32, kind="ExternalInput")
    bc_d = nc.dram_tensor("bc", [128, NB], f32, kind="ExternalInput")
    bvB_d = nc.dram_tensor("bvB", [128, D], bf16, kind="ExternalInput")
    out_d = nc.dram_tensor("outT", [D, S], f32, kind="ExternalOutput")
    rdram = nc.dram_tensor("rbounce", [H, S], f32)

    ExpF = mybir.ActivationFunctionType.Exp

    with tile.TileContext(nc) as tc:
        with tc.tile_pool(name="x", bufs=3) as xp, \
             tc.tile_pool(name="wfull", bufs=1) as wfp, \
             tc.tile_pool(name="wblk", bufs=4) as wbp, \
             tc.tile_pool(name="cst", bufs=1) as cp, \
             tc.tile_pool(name="qk", bufs=4) as qkp, \
             tc.tile_pool(name="pers", bufs=1) as prp, \
             tc.tile_pool(name="pt", bufs=2) as ptp, \
             tc.tile_pool(name="stage", bufs=2) as stp, \
             tc.tile_pool(name="rcp", bufs=2) as rcp, \
             tc.tile_pool(name="ost", bufs=2) as ostp, \
             tc.tile_pool(name="mm", bufs=2, space="PSUM") as mmp, \
             tc.tile_pool(name="vp", bufs=2, space="PSUM") as vpp:

            # ---- loads: first V-proj blocks (interleaved for early start),
            # constants after the first block pair, then xq/xk; wc last.
            xv_t = xp.tile([128, NB, S], bf16, tag="x")
            wv_t = wfp.tile([128, NB, D], bf16)
            nc.sync.dma_start(out=xv_t[:, 0, :], in_=xv_d.ap()[:, 0, :])
            nc.sync.dma_start(out=wv_t[:, 0, :], in_=wv_d.ap()[:, 0, :])
            bq_t = cp.tile([128, NB], f32)
            nc.sync.dma_start(out=bq_t[:, :], in_=bq_d.ap())
            bk_t = cp.tile([128, NB], f32)
            nc.sync.dma_start(out=bk_t[:, :], in_=bk_d.ap())
            bc_t = cp.tile([128, NB], f32)
            nc.sync.dma_start(out=bc_t[:, :], in_=bc_d.ap())
            bvB_t = cp.tile([128, D], bf16)
            nc.sync.dma_start(out=bvB_t[:, :], in_=bvB_d.ap())
            ones64 = cp.tile([65, 64], bf16)
            nc.vector.memset(ones64[:, :], 1.0)
            for n in range(1, NB):
                nc.sync.dma_start(out=xv_t[:, n, :], in_=xv_d.ap()[:, n, :])
                nc.sync.dma_start(out=wv_t[:, n, :], in_=wv_d.ap()[:, n, :])
            xq_t = xp.tile([128, NB, S], bf16, tag="x")
            nc.sync.dma_start(out=xq_t[:, :, :], in_=xq_d.ap())
            xk_t = xp.tile([128, NB, S], bf16, tag="x")
            nc.sync.dma_start(out=xk_t[:, :, :], in_=xk_d.ap())

            v1e = prp.tile([128, NB, H * HW], bf16)
            a1 = [prp.tile([128, S], bf16, tag=f"a1_{n}", name=f"a1_{n}")
                  for n in range(NB)]

            # ones columns of v1e (softmax denominator trick)
            ones_ap = v1e[:, :, :].rearrange("p n (h x) -> p n h x", x=HW)[:, :, :, 64]
            nc.vector.memset(ones_ap, 1.0)

            def vproj_block(n2):
                ps = vpp.tile([128, 2, 512], f32, tag="vp", name=f"vps{n2}")
                for n in range(NB):
                    for c in range(2):
                        nc.tensor.matmul(
                            ps[:, c, :],
                            xv_t[:, n, n2 * 128:(n2 + 1) * 128],
                            wv_t[:, n, c * 512:(c + 1) * 512],
                            start=(n == 0), stop=(n == NB - 1),
                        )
                dst = v1e[:, n2, :].rearrange("p (c h x) -> p c h x", c=2, x=HW)[:, :, :, 0:64]
                ps_v = ps[:, :, :].rearrange("p c (h x) -> p c h x", x=64)
                bv_v = bvB_t[:, :].rearrange("p (c h x) -> p c h x", c=2, x=64)
                nc.vector.tensor_add(dst, ps_v, bv_v)

            def wblk_load(w_d, m):
                wb = wbp.tile([128, NB, 128], bf16, tag="wblk")
                nc.sync.dma_start(out=wb[:, :, :], in_=w_d.ap()[:, m, :, :])
                return wb

            def proj_block(wb, x_t, b_t, m):
                """[e-block m, s] = W.T-block @ x.T (+ bias) -> f32 tile.
                Kept in f32 so the scores matmuls can run in float32r
                (full-rate for moving dim >= 256) for better accuracy."""
                ps = vpp.tile([128, 2, 512], f32, tag="vp")
                for n in range(NB):
                    for c in range(2):
                        nc.tensor.matmul(
                            ps[:, c, :], wb[:, n, :], x_t[:, n, c * 512:(c + 1) * 512],
                            start=(n == 0), stop=(n == NB - 1),
                        )
                ob = qkp.tile([128, S], f32r, tag="qk")
                nc.vector.tensor_scalar_add(
                    ob[:, :].rearrange("p (c s) -> p c s", c=2), ps[:, :, :], b_t[:, m:m + 1])
                return ob

            def head(h, q1b, k1b, defer_norm=False, mid_cb=None):
                po = (h % 2) * 64
                pt = ptp.tile([128, NB, S], bf16, tag="pt")
                vps = vpp.tile([65, 2, 512], f32, tag="vp")

                def pv_chunk(n):
                    for c in range(2):
                        nc.tensor.matmul(
                            vps[:, c, :],
                            v1e[:, n, h * HW:(h + 1) * HW],
                            pt[:, n, c * 512:(c + 1) * 512],
                            start=(n == 0), stop=(n == NB - 1),
                        )

                # interleave PV fill in 2-chunk bursts between scores chunks
                # (fewer PE context switches than per-chunk interleave)
                for n in range(NB):
                    sps = mmp.tile([128, 2, 512], f32, tag="mm")
                    for c in range(2):
                        nc.tensor.matmul(
                            sps[:, c, :],
                            k1b[po:po + 64, n * 128:(n + 1) * 128],
                            q1b[po:po + 64, c * 512:(c + 1) * 512],
                            start=True, stop=True,
                        )
                    nc.scalar.activation(
                        out=pt[:, n, :].rearrange("p (c s) -> p c s", c=2),
                        in_=sps[:, :, :], func=ExpF, scale=0.125)
                    if n in (3, 5, 7):
                        pv_chunk(n - 3)
                        pv_chunk(n - 2)
                if mid_cb is not None:
                    mid_cb()
                for n in range(NB - 2, NB):
                    pv_chunk(n)

                if h < H - 1:
                    # DRAM-bounce partition broadcast of 1/denom (off critical
                    # path for all but the last heads)
                    rc = rcp.tile([65, S], f32, tag="rc")
                    nc.vector.reciprocal(
                        out=rc[64:65, :].rearrange("p (c s) -> p c s", c=2),
                        in_=vps[64:65, :, :])
                    nc.gpsimd.dma_start(out=rdram.ap()[h:h + 1, :], in_=rc[64:65, :])
                    rb = rcp.tile([64, S], f32, tag="rb")
                    nc.gpsimd.dma_start(
                        out=rb[:, :], in_=rdram.ap()[h:h + 1, :].to_broadcast((64, S)))
                else:
                    # last head pair feeds the output projection directly:
                    # use the shorter PE-broadcast chain (K=1 matmul) instead
                    rc = rcp.tile([65, S], f32, tag="rc")
                    nc.vector.reciprocal(
                        out=rc[64:65, :].rearrange("p (c s) -> p c s", c=2),
                        in_=vps[64:65, :, :])
                    rcb = rcp.tile([65, S], bf16, tag="rcb")
                    nc.vector.tensor_copy(rcb[64:65, :], rc[64:65, :])
                    rbp = mmp.tile([64, 2, 512], f32, tag="mm")
                    for c in range(2):
                        nc.tensor.matmul(
                            rbp[:, c, :], ones64[64:65, 0:64],
                            rcb[64:65, c * 512:(c + 1) * 512],
                            start=True, stop=True)
                    rb = rcp.tile([64, S], f32, tag="rb")
                    nc.vector.tensor_copy(
                        rb[:, :].rearrange("p (c s) -> p c s", c=2), rbp[:, :, :])

                m = h // 2

                def finish():
                    if h % 2 == 0:
                        dst = a1[m][0:64, :]
                        st = None
                    else:
                        st = stp.tile([64, S], bf16, tag="st")
                        dst = st[:, :]
                    nc.vector.tensor_mul(
                        dst.rearrange("p (c s) -> p c s", c=2),
                        vps[0:64, :, :],
                        rb[:, :].rearrange("p (c s) -> p c s", c=2))
                    if st is not None:
                        if h == H - 1:
                            nc.sync.dma_start(out=a1[m][64:128, :], in_=st[:, :])
                        else:
                            nc.gpsimd.dma_start(out=a1[m][64:128, :], in_=st[:, :])

                if defer_norm:
                    return finish
                finish()

            # ---- V projection (with Q0/K0 interleaved near the end so their
            # PSUM->SBUF drains hide under the remaining V-proj blocks) ----
            wqb = wblk_load(wq_d, 0)
            wkb = wblk_load(wk_d, 0)
            wc_t = wfp.tile([128, NB, NB, 128], bf16, tag="wc")
            nc.sync.dma_start(out=wc_t[:, :, :, :], in_=wc_d.ap())
            for n2 in range(NB - 2):
                vproj_block(n2)
            q1b = proj_block(wqb, xq_t, bq_t, 0)
            vproj_block(NB - 2)
            k1b = proj_block(wkb, xk_t, bk_t, 0)
            vproj_block(NB - 1)
            for m in range(NB):
                if m < NB - 1:
                    head(2 * m, q1b, k1b)
                    nwqb = wblk_load(wq_d, m + 1)
                    nwkb = wblk_load(wk_d, m + 1)
                    nq1b = proj_block(nwqb, xq_t, bq_t, m + 1)
                    nk1b = proj_block(nwkb, xk_t, bk_t, m + 1)
                    head(2 * m + 1, q1b, k1b)
                    q1b, k1b = nq1b, nk1b
                else:
                    # last pair: defer head-14's normalize multiply so its
                    # DRAM-bounce hides under head-15's scores, then finish it
                    # mid-head-15 (keeps the a1[7] tail chain short)
                    fin14 = head(2 * m, q1b, k1b, defer_norm=True)
                    head(2 * m + 1, q1b, k1b, mid_cb=fin14)

            # ---- output projection ----
            for m in range(NB):
                ops = mmp.tile([128, 2, 512], f32, tag="mm")
                for n in range(NB):
                    for c in range(2):
                        nc.tensor.matmul(
                            ops[:, c, :], wc_t[:, m, n, :], a1[n][:, c * 512:(c + 1) * 512],
                            start=(n == 0), stop=(n == NB - 1),
                        )
                if m < NB - 1:
                    ot = ostp.tile([128, 2, 512], f32, tag="ost")
                    nc.vector.tensor_scalar_add(ot[:, :, :], ops[:, :, :], bc_t[:, m:m + 1])
                    nc.sync.dma_start(
                        out=out_d.ap()[m * 128:(m + 1) * 128, :].rearrange(
                            "p (c s) -> p c s", c=2),
                        in_=ot[:, :, :])
                else:
                    # split the last store so its drain+DMA chain pipelines
                    for c in range(2):
                        ot = ostp.tile([128, 512], f32, tag="ostl")
                        nc.vector.tensor_scalar_add(ot[:, :], ops[:, c, :], bc_t[:, m:m + 1])
                        nc.sync.dma_start(
                            out=out_d.ap()[m * 128:(m + 1) * 128,
                                           c * 512:(c + 1) * 512],
                            in_=ot[:, :])

    nc.compile()
    return nc


def _get_nc():
    global _compiled
    if _compiled is None:
        _compiled = _build()
    return _compiled


def _make_in_maps(q, k, v, Wq, bq, Wk, bk, Wv, bv, Wq2, bq2, Wk2, bk2, Wv2, bv2,
                  Wc, bc, Wc2, bc2):
    bf16 = ml_dtypes.bfloat16

    def xpack(x):  # [s, d] -> [p, n, s]
        x = np.asarray(x, np.float32)
        return np.ascontiguousarray(x.reshape(S, NB, 128).transpose(2, 1, 0)).astype(bf16)

    def wpack(w):  # W[e, d] -> [p, m, n, e']
        w = np.asarray(w, np.float32)
        return np.ascontiguousarray(
            w.reshape(NB, 128, NB, 128).transpose(3, 0, 2, 1)).astype(bf16)

    def wvpack(w):  # Wv[e, d] -> [p, n, e]
        w = np.asarray(w, np.float32)
        return np.ascontiguousarray(w.T.reshape(NB, 128, D).transpose(1, 0, 2)).astype(bf16)

    def btile(b):
        return np.ascontiguousarray(np.asarray(b, np.float32).reshape(NB, 128).T)

    def brep(b):
        return np.ascontiguousarray(
            np.broadcast_to(np.asarray(b, np.float32), (128, D))).astype(bf16)

    paths = [
        dict(wq=wpack(Wq), wk=wpack(Wk), wv=wvpack(Wv), wc=wpack(Wc),
             bq=btile(bq), bk=btile(bk), bc=btile(bc), bvB=brep(bv)),
        dict(wq=wpack(Wq2), wk=wpack(Wk2), wv=wvpack(Wv2), wc=wpack(Wc2),
             bq=btile(bq2), bk=btile(bk2), bc=btile(bc2), bvB=brep(bv2)),
    ]
    in_maps = []
    for c in range(8):
        p, b = c // 4, c % 4
        if p == 0:
            xq, xk, xv = xpack(q[b]), xpack(k[b]), xpack(v[b])
        else:
            # path 2: q2 from k; k2, v2 from q
            xq, xk, xv = xpack(k[b]), xpack(q[b]), xpack(q[b])
        in_maps.append(dict(paths[p], xq=xq, xk=xk, xv=xv))
    return in_maps


def _run(in_maps, trace=False):
    from concourse.bass_utils import run_bass_kernel_spmd
    nc = _get_nc()
    return run_bass_kernel_spmd(nc, in_maps, core_ids=list(range(8)), trace=trace)


def kernel(**inputs):
    in_maps = _make_in_maps(**inputs)
    try:
        res = _run(in_maps)
    except Exception:
        # transient NRT_EXEC_UNIT_UNRECOVERABLE has been observed when a
        # prior process crashed mid-execution; one retry reloads the NEFF
        res = _run(in_maps)
    out1 = np.stack([res.results[b]["outT"].T for b in range(4)]).astype(np.float32)
    out2 = np.stack([res.results[4 + b]["outT"].T for b in range(4)]).astype(np.float32)
    return out1, out2



# revision 5
# speedup vs baseline: 1.2103x; 1.2103x over previous
"""Dual-path multi-head attention on 8 trn2 NeuronCores.

Sharding: core c = (path p=c//4, batch b=c%4). Each core runs the full
pipeline for one path and one batch element: 3 input projections, 16-head
attention (S=1024, dh=64), output projection. No collectives.

Path 2 cross-wiring (q2 from k; k2,v2 from q) is handled purely by host-side
input routing - every core runs the identical SPMD program.

Device layouts (per core, all pre-packed on host for contiguous DMA runs):
  xq/xk/xv : [p, n, s]   = x.T blocked:  x[s, n*128+p]
  wq/wc    : [p, m, n, e'] = W[m*128+e', n*128+p]  (W.T blocked by out-block m)
  wv       : [p, n, e]   = Wv[e, n*128+p]
  Projections compute Q1T/K1T = [e, s] (f32r) and V1 = [s, e]; scores are
  computed transposed (pt[sk, sq] = exp(scores/8), max-free softmax) so the
  probs need no transposes. The PV product is computed in the cheap
  orientation out[sq, dh+1] = pt-block.T @ v1e-headslice (moving free = 65
  instead of 512; the 65th v1e column of ones yields the softmax denominator
  per output row). Each [sq,65] block is normalized with a per-partition
  reciprocal (tensor_scalar), staged per head pair as st[sq, 128], and
  flipped back to the [d, s] layout required by the output projection with an
  XBAR dma-transpose (SBUF->SBUF, no PE/PSUM involvement).
Emission is software-pipelined per head h: scores(h)+exp(h) stream while
PV(h-1) and the next pair's Q/K projection matmuls fill the PE between exp
completions, and dma-transposes of pair (h//2-1) run on the sync queue.
"""

import numpy as np
import ml_dtypes

B, S, D, H, DH = 4, 1024, 1024, 16, 64
NB = D // 128  # 8 partition-blocks
HW = 65  # head slot width in v1e (64 data + 1 ones col)

_compiled = None


def _build():
    import concourse.bass as bass
    import concourse.mybir as mybir
    import concourse.tile as tile
    from concourse import bacc

    dt = mybir.dt
    f32, bf16, f32r = dt.float32, dt.bfloat16, dt.float32r

    nc = bacc.Bacc("TRN2", target_bir_lowering=False, debug=False)

    xq_d = nc.dram_tensor("xq", [128, NB, S], bf16, kind="ExternalInput")
    xk_d = nc.dram_tensor("xk", [128, NB, S], bf16, kind="ExternalInput")
    xv_d = nc.dram_tensor("xv", [128, NB, S], bf16, kind="ExternalInput")
    wq_d = nc.dram_tensor("wq", [128, NB, NB, 128], bf16, kind="ExternalInput")
    wk_d = nc.dram_tensor("wk", [128, NB, NB, 128], bf16, kind="ExternalInput")
    wv_d = nc.dram_tensor("wv", [128, NB, D], bf16, kind="ExternalInput")
    wc_d = nc.dram_tensor("wc", [128, NB, NB, 128], bf16, kind="ExternalInput")
    bq_d = nc.dram_tensor("bq", [128, NB], f32, kind="ExternalInput")
    bk_d = nc.dram_tensor("bk", [128, NB], f32, kind="ExternalInput")
    bc_d = nc.dram_tensor("bc", [128, NB], f32, kind="ExternalInput")
    bvB_d = nc.dram_tensor("bvB", [128, D], bf16, kind="ExternalInput")
    out_d = nc.dram_tensor("outT", [D, S], f32, kind="ExternalOutput")

    ExpF = mybir.ActivationFunctionType.Exp

    with tile.TileContext(nc) as tc:
        with tc.tile_pool(name="x", bufs=3) as xp, \
             tc.tile_pool(name="wfull", bufs=1) as wfp, \
             tc.tile_pool(name="wblk", bufs=4) as wbp, \
             tc.tile_pool(name="cst", bufs=1) as cp, \
             tc.tile_pool(name="qk", bufs=4) as qkp, \
             tc.tile_pool(name="pers", bufs=1) as prp, \
             tc.tile_pool(name="pt", bufs=2) as ptp, \
             tc.tile_pool(name="stage", bufs=24) as stp, \
             tc.tile_pool(name="rcp", bufs=8) as rcp, \
             tc.tile_pool(name="ost", bufs=2) as ostp, \
             tc.tile_pool(name="mm", bufs=2, space="PSUM") as mmp, \
             tc.tile_pool(name="pj", bufs=2, space="PSUM") as pjp, \
             tc.tile_pool(name="pv", bufs=2, space="PSUM") as pvp:

            # ---- input loads (sync queue, in consumption order) ----
            xv_t = xp.tile([128, NB, S], bf16, tag="x")
            wv_t = wfp.tile([128, NB, D], bf16)
            nc.sync.dma_start(out=xv_t[:, 0, :], in_=xv_d.ap()[:, 0, :])
            nc.sync.dma_start(out=wv_t[:, 0, :], in_=wv_d.ap()[:, 0, :])
            for n in range(1, NB):
                nc.sync.dma_start(out=xv_t[:, n, :], in_=xv_d.ap()[:, n, :])
                nc.sync.dma_start(out=wv_t[:, n, :], in_=wv_d.ap()[:, n, :])

            def wblk_load(w_d, m):
                wb = wbp.tile([128, NB, 128], bf16, tag="wblk")
                nc.sync.dma_start(out=wb[:, :, :], in_=w_d.ap()[:, m, :, :])
                return wb

            wqb = {0: wblk_load(wq_d, 0)}
            wkb = {0: wblk_load(wk_d, 0)}
            xq_t = xp.tile([128, NB, S], bf16, tag="x")
            xk_t = xp.tile([128, NB, S], bf16, tag="x")
            for n in range(NB):
                nc.sync.dma_start(out=xq_t[:, n, :], in_=xq_d.ap()[:, n, :])
                nc.sync.dma_start(out=xk_t[:, n, :], in_=xk_d.ap()[:, n, :])
            wqb[1] = wblk_load(wq_d, 1)
            wkb[1] = wblk_load(wk_d, 1)
            wc_t = wfp.tile([128, NB, NB, 128], bf16, tag="wc")
            nc.sync.dma_start(out=wc_t[:, :, :, :], in_=wc_d.ap())

            # constants on the gpsimd queue (tiny; off the sync stream)
            bq_t = cp.tile([128, NB], f32)
            nc.gpsimd.dma_start(out=bq_t[:, :], in_=bq_d.ap())
            bk_t = cp.tile([128, NB], f32)
            nc.gpsimd.dma_start(out=bk_t[:, :], in_=bk_d.ap())
            bc_t = cp.tile([128, NB], f32)
            nc.gpsimd.dma_start(out=bc_t[:, :], in_=bc_d.ap())
            bvB_t = cp.tile([128, D], bf16)
            nc.gpsimd.dma_start(out=bvB_t[:, :], in_=bvB_d.ap())

            v1e = prp.tile([128, NB, H * HW], bf16)
            a1 = [prp.tile([128, S], bf16, tag=f"a1_{n}", name=f"a1_{n}")
                  for n in range(NB)]

            # ones columns of v1e (softmax denominator trick)
            ones_ap = v1e[:, :, :].rearrange("p n (h x) -> p n h x", x=HW)[:, :, :, 64]
            nc.vector.memset(ones_ap, 1.0)

            # ---- V projection: full blocks into "mm" psum tiles ----
            def vproj_block(n2):
                ps = mmp.tile([128, 2, 512], f32, tag="mm", name=f"vps{n2}")
                for n in range(NB):
                    for c in range(2):
                        nc.tensor.matmul(
                            ps[:, c, :],
                            xv_t[:, n, n2 * 128:(n2 + 1) * 128],
                            wv_t[:, n, c * 512:(c + 1) * 512],
                            start=(n == 0), stop=(n == NB - 1),
                        )
                dst = v1e[:, n2, :].rearrange("p (c h x) -> p c h x", c=2, x=HW)[:, :, :, 0:64]
                ps_v = ps[:, :, :].rearrange("p c (h x) -> p c h x", x=64)
                bv_v = bvB_t[:, :].rearrange("p (c h x) -> p c h x", c=2, x=64)
                nc.vector.tensor_add(dst, ps_v, bv_v)

            # ---- Q/K projection as "pj" half-tiles; emitted via a step
            # machine so the matmuls can interleave into the head loop.
            class ProjEmitter:
                """16 MMs + 2 bias-adds over 8 .step() calls (2 MMs each)."""

                def __init__(self, wb, x_t, b_t, m, what):
                    self.wb, self.x_t, self.b_t, self.m = wb, x_t, b_t, m
                    self.ob = qkp.tile([128, S], f32r, tag="qk", name=f"ob_{what}{m}")
                    self.ps = None
                    self.k = 0
                    self.what = what

                def step(self):
                    if self.k >= 16:
                        return
                    for _ in range(2):
                        c, n = self.k // 8, self.k % 8
                        if n == 0:
                            self.ps = pjp.tile(
                                [128, 512], f32, tag="pj",
                                name=f"pj_{self.what}{self.m}_{c}")
                        nc.tensor.matmul(
                            self.ps[:, :], self.wb[:, n, :],
                            self.x_t[:, n, c * 512:(c + 1) * 512],
                            start=(n == 0), stop=(n == NB - 1),
                        )
                        self.k += 1
                        if n == NB - 1:
                            nc.vector.tensor_scalar_add(
                                self.ob[:, c * 512:(c + 1) * 512],
                                self.ps[:, :], self.b_t[:, self.m:self.m + 1])

                def run_all(self):
                    for _ in range(8):
                        self.step()
                    return self.ob

            # V-proj blocks 0..5 up front; 6,7 stream inside head 0.
            for n2 in range(6):
                vproj_block(n2)
            q1 = {0: ProjEmitter(wqb[0], xq_t, bq_t, 0, "q").run_all()}
            k1 = {0: ProjEmitter(wkb[0], xk_t, bk_t, 0, "k").run_all()}

            # vproj 6,7 via "pj" halves: 4 MMs per step over 8 steps
            class VProjTail:
                def __init__(self):
                    self.k = 0
                    self.ps = None

                def step(self):
                    for _ in range(4):
                        blk = 6 + self.k // 16
                        c = (self.k // 8) % 2
                        n = self.k % 8
                        if n == 0:
                            self.ps = pjp.tile([128, 512], f32, tag="pj",
                                               name=f"vpj{blk}_{c}")
                        nc.tensor.matmul(
                            self.ps[:, :],
                            xv_t[:, n, blk * 128:(blk + 1) * 128],
                            wv_t[:, n, c * 512:(c + 1) * 512],
                            start=(n == 0), stop=(n == NB - 1),
                        )
                        self.k += 1
                        if n == NB - 1:
                            dst = v1e[:, blk, :].rearrange(
                                "p (c h x) -> p c h x", c=2, x=HW)[:, c, :, 0:64]
                            ps_v = self.ps[:, :].rearrange("p (h x) -> p h x", x=64)
                            bv_v = bvB_t[:, :].rearrange(
                                "p (c h x) -> p c h x", c=2, x=64)[:, c]
                            nc.vector.tensor_add(dst, ps_v, bv_v)

            vtail = VProjTail()

            # ---- head machinery ----
            pt_tiles = {}
            st_tiles = {}  # st_tiles[m][b] = [sq,128] staging (2 heads' cols)

            def scores_chunk(h, n):
                m = h // 2
                po = (h % 2) * 64
                if n == 0:
                    pt_tiles[h] = ptp.tile([128, NB, S], bf16, tag="pt",
                                           name=f"pt{h}")
                q1b, k1b = q1[m], k1[m]
                sps = mmp.tile([128, 2, 512], f32, tag="mm", name=f"sps{h}_{n}")
                for c in range(2):
                    nc.tensor.matmul(
                        sps[:, c, :],
                        k1b[po:po + 64, n * 128:(n + 1) * 128],
                        q1b[po:po + 64, c * 512:(c + 1) * 512],
                        start=True, stop=True,
                    )
                nc.scalar.activation(
                    out=pt_tiles[h][:, n, :].rearrange("p (c s) -> p c s", c=2),
                    in_=sps[:, :, :], func=ExpF, scale=0.125)

            def pv_group_mms(h, pv_ps, t, bi):
                """8 accumulation MMs for output block b=t*4+bi of head h."""
                b = t * 4 + bi
                ptt = pt_tiles[h]
                for n in range(NB):
                    nc.tensor.matmul(
                        pv_ps[:, bi, :],
                        ptt[:, n, b * 128:(b + 1) * 128],
                        v1e[:, n, h * HW:(h + 1) * HW],
                        start=(n == 0), stop=(n == NB - 1),
                    )

            def pv_drain(h, pv_ps, t):
                """reciprocal of the ones-column + normalize into st."""
                m = h // 2
                col = (h % 2) * 64
                rc = rcp.tile([128, 4], f32, tag="rc", name=f"rc{h}_{t}")
                nc.vector.reciprocal(rc[:, :], pv_ps[:, :, 64])
                if m not in st_tiles:
                    st_tiles[m] = {}
                for bi in range(4):
                    b = t * 4 + bi
                    if b not in st_tiles[m]:
                        st_tiles[m][b] = stp.tile([128, 128], bf16, tag="st",
                                                  name=f"st{m}_{b}")
                    nc.vector.tensor_scalar_mul(
                        st_tiles[m][b][:, col:col + 64],
                        pv_ps[:, bi, 0:64], rc[:, bi:bi + 1])

            # ---- main loop over heads ----
            for h in range(H):
                m = h // 2
                pv_cur = None
                proj = None
                if m + 1 <= NB - 1:
                    tgt = m + 1
                    if h % 2 == 0:
                        proj = ProjEmitter(wqb[tgt], xq_t, bq_t, tgt, "q")
                    else:
                        proj = ProjEmitter(wkb[tgt], xk_t, bk_t, tgt, "k")
                if h % 2 == 1 and (h + 3) // 2 <= NB - 1:
                    p = (h + 3) // 2
                    wqb[p] = wblk_load(wq_d, p)
                    wkb[p] = wblk_load(wk_d, p)
                for n in range(NB):
                    scores_chunk(h, n)
                    if h == 0:
                        vtail.step()
                    if h >= 1:
                        t, bi = n // 4, n % 4
                        if bi == 0:
                            pv_cur = pvp.tile([128, 4, HW], f32, tag="pv",
                                              name=f"pv{h - 1}_{t}")
                        pv_group_mms(h - 1, pv_cur, t, bi)
                        if bi == 3:
                            pv_drain(h - 1, pv_cur, t)
                    if proj is not None:
                        proj.step()
                    if h % 2 == 1 and m >= 1:
                        # flip st(m-1, b=n) into a1[m-1] via XBAR dma-transpose
                        nc.sync.dma_start_transpose(
                            out=a1[m - 1][:, n * 128:(n + 1) * 128],
                            in_=st_tiles[m - 1][n][:, :])
                if proj is not None:
                    if h % 2 == 0:
                        q1[m + 1] = proj.ob
                    else:
                        k1[m + 1] = proj.ob

            # ---- tail: PV of the last head, its transposes ----
            for t in range(2):
                pv_cur = pvp.tile([128, 4, HW], f32, tag="pv", name=f"pv15_{t}")
                for bi in range(4):
                    pv_group_mms(H - 1, pv_cur, t, bi)
                pv_drain(H - 1, pv_cur, t)
            t7 = list(range(NB))  # pending transposes of pair 7

            # ---- output projection ----
            for m in range(NB):
                ops = mmp.tile([128, 2, 512], f32, tag="mm", name=f"ops{m}")
                for n in range(NB):
                    if m == 0 and t7:
                        # interleave pair-7 transposes before a1[7] is needed
                        b = t7.pop(0)
                        nc.sync.dma_start_transpose(
                            out=a1[NB - 1][:, b * 128:(b + 1) * 128],
                            in_=st_tiles[NB - 1][b][:, :])
                        if t7 and n >= 4:
                            b = t7.pop(0)
                            nc.sync.dma_start_transpose(
                                out=a1[NB - 1][:, b * 128:(b + 1) * 128],
                                in_=st_tiles[NB - 1][b][:, :])
                    for c in range(2):
                        nc.tensor.matmul(
                            ops[:, c, :], wc_t[:, m, n, :], a1[n][:, c * 512:(c + 1) * 512],
                            start=(n == 0), stop=(n == NB - 1),
                        )
                if m < NB - 1:
                    ot = ostp.tile([128, 2, 512], f32, tag="ost")
                    nc.vector.tensor_scalar_add(ot[:, :, :], ops[:, :, :], bc_t[:, m:m + 1])
                    nc.sync.dma_start(
                        out=out_d.ap()[m * 128:(m + 1) * 128, :].rearrange(
                            "p (c s) -> p c s", c=2),
                        in_=ot[:, :, :])
                else:
                    # split the last store so its drain+DMA chain pipelines
                    for c in range(2):
                        ot = ostp.tile([128, 512], f32, tag="ostl")
                        nc.vector.tensor_scalar_add(ot[:, :], ops[:, c, :], bc_t[:, m:m + 1])
                        nc.sync.dma_start(
                            out=out_d.ap()[m * 128:(m + 1) * 128,
                                           c * 512:(c + 1) * 512],
                            in_=ot[:, :])

    nc.compile()
    return nc


def _get_nc():
    global _compiled
    if _compiled is None:
        _compiled = _build()
    return _compiled


def _make_in_maps(q, k, v, Wq, bq, Wk, bk, Wv, bv, Wq2, bq2, Wk2, bk2, Wv2, bv2,
                  Wc, bc, Wc2, bc2):
    bf16 = ml_dtypes.bfloat16

    def xpack(x):  # [s, d] -> [p, n, s]
        x = np.asarray(x, np.float32)
        return np.ascontiguousarray(x.reshape(S, NB, 128).transpose(2, 1, 0)).astype(bf16)

    def wpack(w):  # W[e, d] -> [p, m, n, e']
        w = np.asarray(w, np.float32)
        return np.ascontiguousarray(
            w.reshape(NB, 128, NB, 128).transpose(3, 0, 2, 1)).astype(bf16)

    def wvpack(w):  # Wv[e, d] -> [p, n, e]
        w = np.asarray(w, np.float32)
        return np.ascontiguousarray(w.T.reshape(NB, 128, D).transpose(1, 0, 2)).astype(bf16)

    def btile(b):
        return np.ascontiguousarray(np.asarray(b, np.float32).reshape(NB, 128).T)

    def brep(b):
        return np.ascontiguousarray(
            np.broadcast_to(np.asarray(b, np.float32), (128, D))).astype(bf16)

    paths = [
        dict(wq=wpack(Wq), wk=wpack(Wk), wv=wvpack(Wv), wc=wpack(Wc),
             bq=btile(bq), bk=btile(bk), bc=btile(bc), bvB=brep(bv)),
        dict(wq=wpack(Wq2), wk=wpack(Wk2), wv=wvpack(Wv2), wc=wpack(Wc2),
             bq=btile(bq2), bk=btile(bk2), bc=btile(bc2), bvB=brep(bv2)),
    ]
    in_maps = []
    for c in range(8):
        p, b = c // 4, c % 4
        if p == 0:
            xq, xk, xv = xpack(q[b]), xpack(k[b]), xpack(v[b])
        else:
            # path 2: q2 from k; k2, v2 from q
            xq, xk, xv = xpack(k[b]), xpack(q[b]), xpack(q[b])
        in_maps.append(dict(paths[p], xq=xq, xk=xk, xv=xv))
    return in_maps


def _run(in_maps, trace=False):
    from concourse.bass_utils import run_bass_kernel_spmd
    nc = _get_nc()
    return run_bass_kernel_spmd(nc, in_maps, core_ids=list(range(8)), trace=trace)


def kernel(**inputs):
    in_maps = _make_in_maps(**inputs)
    try:
        res = _run(in_maps)
    except Exception:
        # transient NRT_EXEC_UNIT_UNRECOVERABLE has been observed when a
        # prior process crashed mid-execution; one retry reloads the NEFF
        res = _run(in_maps)
    out1 = np.stack([res.results[b]["outT"].T for b in range(4)]).astype(np.float32)
    out2 = np.stack([res.results[4 + b]["outT"].T for b in range(4)]).astype(np.float32)
    return out1, out2
